# revision 1
# baseline (speedup 1.0000x reference)
"""PointGNNConv (sum aggregation) on 8 Trainium2 NeuronCores.

Algebraic decomposition: with f_w = [f_w3; f_wx] (3+128 rows),
    msg_e = relu(edge_feat @ f_w + f_b)
          = relu(u[src_e] + v[dst_e])
    u_j = pos_j @ f_w3 + x_j @ f_wx
    v_i = (delta_i - pos_i) @ f_w3 + f_b
so the per-edge work reduces to two row gathers + add + relu, followed by a
segment-sum (edges sorted by dst).

Sharding: dst-range sharding — core c owns dst in [c*NPC, (c+1)*NPC).
Two NEFFs: phase A computes per-node u/v on each core's node slice (host
reassembles the full u table between NEFFs — pure row reordering, no FP);
phase B gathers u[src]/v[dst] per edge with the SWDGE dma_gather, applies
add+relu, segment-sums via per-chunk matmuls against on-chip-built selection
matrices, applies the g-MLP and residual.

Segment-sum: edges sorted by local dst, grouped into sections of 128 dsts;
per section a fixed number of 128-edge chunks (lo: src < 32768, hi: rest —
the SWDGE gather index is int16 so the u table is addressed as two halves).
Chunk matmul: out[feat, w] += msg_chunk[slot, feat]^T @ S_chunk[slot, w]
with S[slot, w] = (dst_local_in_section[slot] == w), accumulated in PSUM
over a supergroup of 4 sections (512 dsts), then evacuated to an aggT tile.
"""
import sys

sys.path.insert(0, "/opt/trn_rl_repo")

import numpy as np
import ml_dtypes

import concourse.bass as bass
import concourse.mybir as mybir
import concourse.tile as tile
from concourse import bacc
from concourse.bass_utils import run_bass_kernel_spmd

BF16 = mybir.dt.bfloat16
F32 = mybir.dt.float32
I16 = mybir.dt.int16
AF = mybir.ActivationFunctionType
ALU = mybir.AluOpType

NCORES = 8
PAD_PDL = 200.0  # pdl value for pad slots; never equals a real column id


class Cfg:
    def __init__(self, n, e, din, lo_rows, dt=512):
        self.N = n
        self.E = e
        self.DIN = din
        self.NPC = n // NCORES          # nodes (dsts) per core
        self.LO = lo_rows               # u-table split point (int16 idx limit)
        self.SEC = 128                  # dsts per section
        self.NSEC = -(-self.NPC // self.SEC)
        self.SG_SECS = 4                # sections per supergroup (psum window)
        self.NSG = -(-self.NSEC // self.SG_SECS)
        self.DT = dt                    # free-dim tile for phase A / C


def _dtiles(total, dt):
    return [(i, min(dt, total - i)) for i in range(0, total, dt)]


def _secs_of_sg(cfg, sg):
    s0 = sg * cfg.SG_SECS
    return s0, min(s0 + cfg.SG_SECS, cfg.NSEC)


# ---------------------------------------------------------------- phase A
def build_phase_a(cfg):
    nc = bacc.Bacc(num_devices=NCORES)
    D = cfg.DIN
    xT = nc.dram_tensor("xT", [D, cfg.NPC], BF16, kind="ExternalInput")
    posT = nc.dram_tensor("posT", [3, cfg.NPC], BF16, kind="ExternalInput")
    h_w1 = nc.dram_tensor("h_w1", [D, D], BF16, kind="ExternalInput")
    h_b1 = nc.dram_tensor("h_b1", [D, 1], F32, kind="ExternalInput")
    h_w2 = nc.dram_tensor("h_w2", [D, 3], BF16, kind="ExternalInput")
    h_b2 = nc.dram_tensor("h_b2", [3, 1], F32, kind="ExternalInput")
    f_w3 = nc.dram_tensor("f_w3", [3, D], BF16, kind="ExternalInput")
    f_wx = nc.dram_tensor("f_wx", [D, D], BF16, kind="ExternalInput")
    f_b = nc.dram_tensor("f_b", [D, 1], F32, kind="ExternalInput")
    uT = nc.dram_tensor("uT", [D, cfg.NPC], BF16, kind="ExternalOutput")
    vT = nc.dram_tensor("vT", [D, cfg.NPC], BF16, kind="ExternalOutput")

    with tile.TileContext(nc) as tc:
        with (
            tc.tile_pool(name="consts", bufs=1) as cp,
            tc.tile_pool(name="work", bufs=2) as wp,
            tc.tile_pool(name="psum", bufs=2, space="PSUM") as pp,
        ):
            xT_sb = cp.tile([D, cfg.NPC], BF16)
            nc.sync.dma_start(out=xT_sb[:], in_=xT[:])
            posT_sb = cp.tile([3, cfg.NPC], BF16)
            nc.sync.dma_start(out=posT_sb[:], in_=posT[:])
            w1_sb = cp.tile([D, D], BF16)
            nc.sync.dma_start(out=w1_sb[:], in_=h_w1[:])
            w2_sb = cp.tile([D, 3], BF16)
            nc.sync.dma_start(out=w2_sb[:], in_=h_w2[:])
            fw3_sb = cp.tile([3, D], BF16)
            nc.sync.dma_start(out=fw3_sb[:], in_=f_w3[:])
            fwx_sb = cp.tile([D, D], BF16)
            nc.sync.dma_start(out=fwx_sb[:], in_=f_wx[:])
            b1_sb = cp.tile([D, 1], F32)
            nc.sync.dma_start(out=b1_sb[:], in_=h_b1[:])
            b2_sb = cp.tile([3, 1], F32)
            nc.sync.dma_start(out=b2_sb[:], in_=h_b2[:])
            fb_sb = cp.tile([D, 1], F32)
            nc.sync.dma_start(out=fb_sb[:], in_=f_b[:])

            for off, w in _dtiles(cfg.NPC, cfg.DT):
                sl = slice(off, off + w)
                xt = xT_sb[:, sl]
                pt = posT_sb[:, sl]
                # t1 = relu(x @ h_w1 + h_b1)   [D, w]
                ps1 = pp.tile([D, cfg.DT], F32, tag="ps1")
                nc.tensor.matmul(out=ps1[:, :w], lhsT=w1_sb[:], rhs=xt,
                                 start=True, stop=True)
                t1 = wp.tile([D, cfg.DT], BF16, tag="t1")
                nc.scalar.activation(out=t1[:, :w], in_=ps1[:, :w], func=AF.Relu,
                                     bias=b1_sb[:])
                # delta = tanh(t1 @ h_w2 + h_b2)  [3, w]
                ps2 = pp.tile([3, cfg.DT], F32, tag="ps2")
                nc.tensor.matmul(out=ps2[:, :w], lhsT=w2_sb[:], rhs=t1[:, :w],
                                 start=True, stop=True)
                dmp = wp.tile([3, cfg.DT], BF16, tag="dmp")
                nc.scalar.activation(out=dmp[:, :w], in_=ps2[:, :w], func=AF.Tanh,
                                     bias=b2_sb[:])
                # diff = delta - pos  [3, w]
                nc.vector.tensor_tensor(out=dmp[:, :w], in0=dmp[:, :w],
                                        in1=pt, op=ALU.subtract)
                # u = pos @ f_w3 + x @ f_wx   [D, w]
                psu = pp.tile([D, cfg.DT], F32, tag="psu")
                nc.tensor.matmul(out=psu[:, :w], lhsT=fw3_sb[:], rhs=pt,
                                 start=True, stop=False)
                nc.tensor.matmul(out=psu[:, :w], lhsT=fwx_sb[:], rhs=xt,
                                 start=False, stop=True)
                ut = wp.tile([D, cfg.DT], BF16, tag="ut")
                nc.scalar.activation(out=ut[:, :w], in_=psu[:, :w], func=AF.Copy)
                nc.sync.dma_start(out=uT[:, sl], in_=ut[:, :w])
                # v = (delta - pos) @ f_w3 + f_b  [D, w]
                psv = pp.tile([D, cfg.DT], F32, tag="psv")
                nc.tensor.matmul(out=psv[:, :w], lhsT=fw3_sb[:], rhs=dmp[:, :w],
                                 start=True, stop=True)
                vt = wp.tile([D, cfg.DT], BF16, tag="vt")
                nc.scalar.activation(out=vt[:, :w], in_=psv[:, :w],
                                     func=AF.Identity, bias=fb_sb[:])
                nc.sync.dma_start(out=vT[:, sl], in_=vt[:, :w])
    nc.finalize()
    return nc


# ---------------------------------------------------------------- phase B
def build_phase_b(cfg, c_lo, c_hi):
    nc = bacc.Bacc(num_devices=NCORES)
    D = cfg.DIN
    CLH = c_lo + c_hi
    lo_cols = cfg.NSEC * c_lo * 8   # idx cols (16 idx per col) for lo stream
    hi_cols = cfg.NSEC * c_hi * 8

    u_tbl = nc.dram_tensor("u_tbl", [cfg.N, D], BF16, kind="ExternalInput")
    v_tbl = nc.dram_tensor("v_tbl", [cfg.NPC, D], BF16, kind="ExternalInput")
    xT = nc.dram_tensor("xTf", [D, cfg.NPC], F32, kind="ExternalInput")
    uidx_lo = nc.dram_tensor("uidx_lo", [128, lo_cols], I16, kind="ExternalInput")
    vidx_lo = nc.dram_tensor("vidx_lo", [128, lo_cols], I16, kind="ExternalInput")
    if c_hi:
        uidx_hi = nc.dram_tensor("uidx_hi", [128, hi_cols], I16,
                                 kind="ExternalInput")
        vidx_hi = nc.dram_tensor("vidx_hi", [128, hi_cols], I16,
                                 kind="ExternalInput")
    pdl = nc.dram_tensor("pdl", [128, cfg.NSEC * CLH], BF16, kind="ExternalInput")
    gw1 = nc.dram_tensor("gw1", [D, D], BF16, kind="ExternalInput")
    gb1 = nc.dram_tensor("gb1", [D, 1], F32, kind="ExternalInput")
    gw2 = nc.dram_tensor("gw2", [D, D], BF16, kind="ExternalInput")
    gb2 = nc.dram_tensor("gb2", [D, 1], F32, kind="ExternalInput")
    outT = nc.dram_tensor("outT", [D, cfg.NPC], F32, kind="ExternalOutput")

    iota = nc.inline_tensor(
        np.broadcast_to(np.arange(128, dtype=ml_dtypes.bfloat16), (128, 128)).copy(),
        name="iota",
    )

    agg_cols = cfg.NSEC * cfg.SEC  # padded agg width (multiple of 128)

    with tile.TileContext(nc) as tc:
        with (
            tc.tile_pool(name="consts", bufs=1) as cp,
            tc.tile_pool(name="gat", bufs=2) as gp,
            tc.tile_pool(name="sbld", bufs=2) as sp,
            tc.tile_pool(name="cwork", bufs=2) as wp,
            tc.tile_pool(name="psagg", bufs=2, space="PSUM") as pa,
            tc.tile_pool(name="psmlp", bufs=2, space="PSUM") as pm,
        ):
            iota_sb = cp.tile([128, 128], BF16)
            nc.sync.dma_start(out=iota_sb[:], in_=iota[:])
            pdl_sb = cp.tile([128, cfg.NSEC * CLH], BF16)
            nc.sync.dma_start(out=pdl_sb[:], in_=pdl[:])
            uilo_sb = cp.tile([128, lo_cols], I16)
            nc.sync.dma_start(out=uilo_sb[:], in_=uidx_lo[:])
            vilo_sb = cp.tile([128, lo_cols], I16)
            nc.sync.dma_start(out=vilo_sb[:], in_=vidx_lo[:])
            if c_hi:
                uihi_sb = cp.tile([128, hi_cols], I16)
                nc.sync.dma_start(out=uihi_sb[:], in_=uidx_hi[:])
                vihi_sb = cp.tile([128, hi_cols], I16)
                nc.sync.dma_start(out=vihi_sb[:], in_=vidx_hi[:])
            gw1_sb = cp.tile([D, D], BF16)
            nc.sync.dma_start(out=gw1_sb[:], in_=gw1[:])
            gw2_sb = cp.tile([D, D], BF16)
            nc.sync.dma_start(out=gw2_sb[:], in_=gw2[:])
            gb1_sb = cp.tile([D, 1], F32)
            nc.sync.dma_start(out=gb1_sb[:], in_=gb1[:])
            gb2_sb = cp.tile([D, 1], F32)
            nc.sync.dma_start(out=gb2_sb[:], in_=gb2[:])
            aggT = cp.tile([D, agg_cols], BF16)

            for sg in range(cfg.NSG):
                s0, s1 = _secs_of_sg(cfg, sg)
                secs = s1 - s0
                nlo = secs * c_lo * 128
                nhi = secs * c_hi * 128

                ulo = gp.tile([128, cfg.SG_SECS * c_lo, D], BF16, tag="ulo")
                nc.gpsimd.dma_gather(
                    ulo[:, :secs * c_lo, :], u_tbl[:cfg.LO, :],
                    uilo_sb[:, s0 * c_lo * 8:s1 * c_lo * 8], nlo, nlo, D,
                    single_packet=False)
                vlo = gp.tile([128, cfg.SG_SECS * c_lo, D], BF16, tag="vlo")
                nc.gpsimd.dma_gather(
                    vlo[:, :secs * c_lo, :], v_tbl[:],
                    vilo_sb[:, s0 * c_lo * 8:s1 * c_lo * 8], nlo, nlo, D,
                    single_packet=False)
                if c_hi:
                    uhi = gp.tile([128, cfg.SG_SECS * c_hi, D], BF16, tag="uhi")
                    nc.gpsimd.dma_gather(
                        uhi[:, :secs * c_hi, :], u_tbl[cfg.LO:, :],
                        uihi_sb[:, s0 * c_hi * 8:s1 * c_hi * 8], nhi, nhi, D,
                        single_packet=False)
                    vhi = gp.tile([128, cfg.SG_SECS * c_hi, D], BF16, tag="vhi")
                    nc.gpsimd.dma_gather(
                        vhi[:, :secs * c_hi, :], v_tbl[:],
                        vihi_sb[:, s0 * c_hi * 8:s1 * c_hi * 8], nhi, nhi, D,
                        single_packet=False)

                # msg = relu(u + v), in place in the u tiles
                mlo = ulo[:, :secs * c_lo, :].rearrange("p c f -> p (c f)")
                nc.vector.tensor_tensor(
                    out=mlo, in0=mlo,
                    in1=vlo[:, :secs * c_lo, :].rearrange("p c f -> p (c f)"),
                    op=ALU.add)
                nc.vector.tensor_relu(mlo, mlo)
                if c_hi:
                    mhi = uhi[:, :secs * c_hi, :].rearrange("p c f -> p (c f)")
                    nc.vector.tensor_tensor(
                        out=mhi, in0=mhi,
                        in1=vhi[:, :secs * c_hi, :].rearrange("p c f -> p (c f)"),
                        op=ALU.add)
                    nc.vector.tensor_relu(mhi, mhi)

                # selection matrices for every chunk of this supergroup
                st = sp.tile([128, cfg.SG_SECS * CLH, 128], BF16, tag="st")
                for k in range(secs * CLH):
                    col = s0 * CLH + k
                    nc.vector.tensor_tensor(
                        out=st[:, k, :],
                        in0=pdl_sb[:, col:col + 1].to_broadcast([128, 128]),
                        in1=iota_sb[:],
                        op=ALU.is_equal)

                # segment-sum: psum[feat, w] += msg^T @ S per chunk
                ps = pa.tile([D, cfg.SG_SECS * cfg.SEC], F32, tag="psagg")
                for j in range(secs):
                    osl = slice(j * cfg.SEC, (j + 1) * cfg.SEC)
                    for t in range(c_lo):
                        nc.tensor.matmul(
                            out=ps[:, osl],
                            lhsT=ulo[:, j * c_lo + t, :],
                            rhs=st[:, j * CLH + t, :],
                            start=(t == 0), stop=(c_hi == 0 and t == c_lo - 1))
                    for t in range(c_hi):
                        nc.tensor.matmul(
                            out=ps[:, osl],
                            lhsT=uhi[:, j * c_hi + t, :],
                            rhs=st[:, j * CLH + c_lo + t, :],
                            start=False, stop=(t == c_hi - 1))
                nc.scalar.activation(
                    out=aggT[:, s0 * cfg.SEC:s1 * cfg.SEC],
                    in_=ps[:, :secs * cfg.SEC], func=AF.Copy)

            # phase C: out = x + relu(relu(agg @ g_w1 + g_b1) @ g_w2 + g_b2)
            for off, w in _dtiles(cfg.NPC, cfg.DT):
                sl = slice(off, off + w)
                ph1 = pm.tile([D, cfg.DT], F32, tag="ph1")
                nc.tensor.matmul(out=ph1[:, :w], lhsT=gw1_sb[:],
                                 rhs=aggT[:, sl], start=True, stop=True)
                h1 = wp.tile([D, cfg.DT], BF16, tag="h1")
                nc.scalar.activation(out=h1[:, :w], in_=ph1[:, :w], func=AF.Relu,
                                     bias=gb1_sb[:])
                ph2 = pm.tile([D, cfg.DT], F32, tag="ph2")
                nc.tensor.matmul(out=ph2[:, :w], lhsT=gw2_sb[:],
                                 rhs=h1[:, :w], start=True, stop=True)
                h2 = wp.tile([D, cfg.DT], F32, tag="h2")
                nc.scalar.activation(out=h2[:, :w], in_=ph2[:, :w], func=AF.Relu,
                                     bias=gb2_sb[:])
                xt = wp.tile([D, cfg.DT], F32, tag="xt")
                nc.sync.dma_start(out=xt[:, :w], in_=xT[:, sl])
                nc.vector.tensor_tensor(out=h2[:, :w], in0=h2[:, :w],
                                        in1=xt[:, :w], op=ALU.add)
                nc.sync.dma_start(out=outT[:, sl], in_=h2[:, :w])
    nc.finalize()
    return nc


# ------------------------------------------------------------ host side
def _wrap_idx(vals):
    """[n] int16 -> [128, n/16] wrapped (16 partitions) + replicated x8."""
    a = np.asarray(vals, dtype=np.int16).reshape(-1, 16).T  # [16, n/16]
    return np.ascontiguousarray(np.tile(a, (8, 1)))


def _preprocess(cfg, edge_index):
    """Sort/bucket edges per core; build idx + pdl arrays.

    Returns (c_lo, c_hi, per_core list of dicts).
    """
    src = np.asarray(edge_index[0], dtype=np.int64)
    dst = np.asarray(edge_index[1], dtype=np.int64)
    order = np.argsort(dst, kind="stable")
    src, dst = src[order], dst[order]
    core = dst // cfg.NPC
    bounds = np.searchsorted(core, np.arange(NCORES + 1))

    per_core = []
    for c in range(NCORES):
        lo_, hi_ = bounds[c], bounds[c + 1]
        s, d = src[lo_:hi_], dst[lo_:hi_] - c * cfg.NPC
        sec = d // cfg.SEC
        is_lo = s < cfg.LO
        per_core.append((s, d, sec, is_lo))

    def sec_counts(c, want_lo):
        s, d, sec, is_lo = per_core[c]
        m = is_lo if want_lo else ~is_lo
        return np.bincount(sec[m], minlength=cfg.NSEC)

    c_lo = max(1, max(int(np.max(np.ceil(sec_counts(c, True) / 128)))
                      for c in range(NCORES)))
    has_hi = cfg.LO < cfg.N
    c_hi = (max(1, max(int(np.max(np.ceil(sec_counts(c, False) / 128)))
                       for c in range(NCORES)))) if has_hi else 0

    data = []
    for c in range(NCORES):
        s, d, sec, is_lo = per_core[c]
        CLH = c_lo + c_hi
        pdl = np.full((128, cfg.NSEC * CLH), PAD_PDL, np.float32)

        def build(mask, cap, tbl_off):
            uidx = np.zeros((cfg.NSEC, cap * 128), np.int16)
            vidx = np.zeros((cfg.NSEC, cap * 128), np.int16)
            pcol = np.full((cfg.NSEC, cap, 128), PAD_PDL, np.float32)
            ss, dd, qq = s[mask], d[mask], sec[mask]
            for j in range(cfg.NSEC):
                m = qq == j
                n = int(m.sum())
                assert n <= cap * 128
                uidx[j, :n] = (ss[m] - tbl_off).astype(np.int16)
                vidx[j, :n] = dd[m].astype(np.int16)
                loc = (dd[m] - j * cfg.SEC).astype(np.float32)
                flat = pcol[j].reshape(-1)
                flat[:n] = loc
            return uidx, vidx, pcol

        ulo, vlo, plo = build(is_lo, c_lo, 0)
        if c_hi:
            uhi, vhi, phi = build(~is_lo, c_hi, cfg.LO)
        # pdl layout: per section: c_lo lo chunks then c_hi hi chunks;
        # chunk t of section j -> column j*CLH + t; rows = slots
        for j in range(cfg.NSEC):
            pdl[:, j * CLH:j * CLH + c_lo] = plo[j].T.reshape(128, c_lo)
            if c_hi:
                pdl[:, j * CLH + c_lo:(j + 1) * CLH] = phi[j].T.reshape(128, c_hi)

        entry = {
            "uidx_lo": _wrap_idx(ulo.reshape(-1)),
            "vidx_lo": _wrap_idx(vlo.reshape(-1)),
            "pdl": pdl.astype(ml_dtypes.bfloat16),
        }
        if c_hi:
            entry["uidx_hi"] = _wrap_idx(uhi.reshape(-1))
            entry["vidx_hi"] = _wrap_idx(vhi.reshape(-1))
        data.append(entry)
    return c_lo, c_hi, data


def run(cfg, inputs, trace=False):
    """Full pipeline. inputs: dict as from setup_inputs (numpy)."""
    bf = ml_dtypes.bfloat16
    x = np.asarray(inputs["x"], np.float32)
    pos = np.asarray(inputs["pos"], np.float32)
    c_lo, c_hi, edata = _preprocess(cfg, np.asarray(inputs["edge_index"]))

    f_w = np.asarray(inputs["f_w"], np.float32) if "f_w" in inputs else None

    h_w1 = np.asarray(inputs["h_w1"], np.float32)
    h_b1 = np.asarray(inputs["h_b1"], np.float32)
    h_w2 = np.asarray(inputs["h_w2"], np.float32)
    h_b2 = np.asarray(inputs["h_b2"], np.float32)
    f_w = np.asarray(inputs["f_w"], np.float32)
    f_b = np.asarray(inputs["f_b"], np.float32)
    g_w1 = np.asarray(inputs["g_w1"], np.float32)
    g_b1 = np.asarray(inputs["g_b1"], np.float32)
    g_w2 = np.asarray(inputs["g_w2"], np.float32)
    g_b2 = np.asarray(inputs["g_b2"], np.float32)

    nc_a = build_phase_a(cfg)
    in_a = []
    for c in range(NCORES):
        sl = slice(c * cfg.NPC, (c + 1) * cfg.NPC)
        in_a.append({
            "xT": np.ascontiguousarray(x[sl].T.astype(bf)),
            "posT": np.ascontiguousarray(pos[sl].T.astype(bf)),
            "h_w1": h_w1.astype(bf), "h_b1": h_b1[:, None],
            "h_w2": h_w2.astype(bf), "h_b2": h_b2[:, None],
            "f_w3": f_w[:3].astype(bf), "f_wx": f_w[3:].astype(bf),
            "f_b": f_b[:, None],
        })
    res_a = run_bass_kernel_spmd(nc_a, in_a, core_ids=list(range(NCORES)),
                                 trace=trace)
    u_tbl = np.concatenate(
        [np.ascontiguousarray(r["uT"].T) for r in res_a.results], axis=0)
    v_tbls = [np.ascontiguousarray(r["vT"].T) for r in res_a.results]

    nc_b = build_phase_b(cfg, c_lo, c_hi)
    in_b = []
    for c in range(NCORES):
        sl = slice(c * cfg.NPC, (c + 1) * cfg.NPC)
        m = {
            "u_tbl": u_tbl, "v_tbl": v_tbls[c],
            "xTf": np.ascontiguousarray(x[sl].T),
            "gw1": g_w1.astype(bf), "gb1": g_b1[:, None],
            "gw2": g_w2.astype(bf), "gb2": g_b2[:, None],
        }
        m.update(edata[c])
        in_b.append(m)
    res_b = run_bass_kernel_spmd(nc_b, in_b, core_ids=list(range(NCORES)),
                                 trace=trace)
    out = np.concatenate(
        [np.ascontiguousarray(r["outT"].T) for r in res_b.results], axis=0)
    return out, (res_a, res_b)


DEFAULT_CFG = Cfg(n=50000, e=500000, din=128, lo_rows=32768)


def kernel(**inputs):
    out, _ = run(DEFAULT_CFG, inputs)
    return out.astype(np.float32)



# revision 4
# speedup vs baseline: 3.2481x; 3.2481x over previous
"""PointGNNConv (sum aggregation) on 8 Trainium2 NeuronCores.

Algebraic decomposition: with f_w = [f_w3; f_wx] (3+128 rows),
    msg_e = relu(edge_feat @ f_w + f_b) = relu(u[src_e] + v[dst_e])
    u_j = pos_j @ f_w3 + x_j @ f_wx
    v_i = (delta_i - pos_i) @ f_w3 + f_b

Sharding: dst-range sharding - core c owns dst in [c*NPC, (c+1)*NPC).
Two NEFFs: phase A computes per-node u/v on each core's node slice; the
host reassembles per-core u tables between NEFFs (pure row reordering,
no FP); phase B gathers u[src] per edge (SWDGE dma_gather), adds v,
relu, segment-sums, applies the g-MLP and residual.

Identity-chunk layout: each core's nodes are relabeled so nodes of
similar in-degree share a section of 128 (degree sort). Edges are laid
out in chunks of 128 slots where slot s of every chunk of section j
holds an edge with (new) dst j*128+s; chunk k holds each node's k-th
incoming edge. Pad slots point at a table row holding -1e30 so
relu(u_pad + v) == 0. With this layout v is never gathered (the same
v_sec[128, D] tile is reused for every chunk of its section) and the
segment sum is a chunk accumulation on the PE with a constant identity
stationary. Per-section chunk counts are shared across cores (max) so
one SPMD program serves all 8.

Gather: SWDGE dma_gather with int16 indices (signed, so only 32768
rows are addressable per call). Each core gets its OWN u table, whose
rows are ordered by first use (with duplicates when a row falls out of
the 32768-row sliding window), so each gather group reads from the
window u_tbl[W_g : W_g+32768]. Group append counts are padded to the
cross-core max so the window bases W_g are core-independent. Gather
calls are capped at 4096 indices and spread round-robin over SWDGE
queues 1..3: those queues dispatch asynchronously (~100 ns on Pool)
and their descriptor generation runs concurrently, overlapping the
drain; queue 0 would hold the Pool engine for the whole call.
"""
import sys

sys.path.insert(0, "/opt/trn_rl_repo")

import numpy as np
import ml_dtypes

import concourse.bass as bass
import concourse.mybir as mybir
import concourse.tile as tile
from concourse import bacc
from concourse.bass_utils import run_bass_kernel_spmd

BF16 = mybir.dt.bfloat16
F32 = mybir.dt.float32
I16 = mybir.dt.int16
AF = mybir.ActivationFunctionType
ALU = mybir.AluOpType

NCORES = 8
PAD_VAL = -1e30    # u-table pad row value; relu(PAD + v) == 0
WIN = 32768        # dma_gather int16 index window
QSET = (1, 2, 3)   # SWDGE queues for gather calls (0 blocks Pool)


class Cfg:
    def __init__(self, n, e, din, dt=512, chunk_budget=32):
        self.N = n
        self.E = e
        self.DIN = din
        self.NPC = n // NCORES            # nodes (dsts) per core
        self.SEC = 128                    # dsts per section
        self.NSEC = -(-self.NPC // self.SEC)
        self.NW = self.NSEC * self.SEC    # padded node-column count
        self.DT = dt                      # free-dim tile for phase A / C
        self.CB = chunk_budget            # max chunks (x128 idx) per gather


def _dtiles(total, dt):
    return [(i, min(dt, total - i)) for i in range(0, total, dt)]


# ---------------------------------------------------------------- phase A
def build_phase_a(cfg):
    nc = bacc.Bacc(num_devices=NCORES)
    D = cfg.DIN
    xT = nc.dram_tensor("xT", [D, cfg.NPC], BF16, kind="ExternalInput")
    posT = nc.dram_tensor("posT", [3, cfg.NPC], BF16, kind="ExternalInput")
    h_w1 = nc.dram_tensor("h_w1", [D, D], BF16, kind="ExternalInput")
    h_b1 = nc.dram_tensor("h_b1", [D, 1], F32, kind="ExternalInput")
    h_w2 = nc.dram_tensor("h_w2", [D, 3], BF16, kind="ExternalInput")
    h_b2 = nc.dram_tensor("h_b2", [3, 1], F32, kind="ExternalInput")
    f_w3 = nc.dram_tensor("f_w3", [3, D], BF16, kind="ExternalInput")
    f_wx = nc.dram_tensor("f_wx", [D, D], BF16, kind="ExternalInput")
    f_b = nc.dram_tensor("f_b", [D, 1], F32, kind="ExternalInput")
    uT = nc.dram_tensor("uT", [D, cfg.NPC], BF16, kind="ExternalOutput")
    vT = nc.dram_tensor("vT", [D, cfg.NPC], BF16, kind="ExternalOutput")

    with tile.TileContext(nc) as tc:
        with (
            tc.tile_pool(name="consts", bufs=1) as cp,
            tc.tile_pool(name="work", bufs=2) as wp,
            tc.tile_pool(name="psum", bufs=2, space="PSUM") as pp,
        ):
            xT_sb = cp.tile([D, cfg.NPC], BF16)
            nc.sync.dma_start(out=xT_sb[:], in_=xT[:])
            posT_sb = cp.tile([3, cfg.NPC], BF16)
            nc.sync.dma_start(out=posT_sb[:], in_=posT[:])
            w1_sb = cp.tile([D, D], BF16)
            nc.sync.dma_start(out=w1_sb[:], in_=h_w1[:])
            w2_sb = cp.tile([D, 3], BF16)
            nc.sync.dma_start(out=w2_sb[:], in_=h_w2[:])
            fw3_sb = cp.tile([3, D], BF16)
            nc.sync.dma_start(out=fw3_sb[:], in_=f_w3[:])
            fwx_sb = cp.tile([D, D], BF16)
            nc.sync.dma_start(out=fwx_sb[:], in_=f_wx[:])
            b1_sb = cp.tile([D, 1], F32)
            nc.sync.dma_start(out=b1_sb[:], in_=h_b1[:])
            b2_sb = cp.tile([3, 1], F32)
            nc.sync.dma_start(out=b2_sb[:], in_=h_b2[:])
            fb_sb = cp.tile([D, 1], F32)
            nc.sync.dma_start(out=fb_sb[:], in_=f_b[:])

            for off, w in _dtiles(cfg.NPC, cfg.DT):
                sl = slice(off, off + w)
                xt = xT_sb[:, sl]
                pt = posT_sb[:, sl]
                # t1 = relu(x @ h_w1 + h_b1)   [D, w]
                ps1 = pp.tile([D, cfg.DT], F32, tag="ps1")
                nc.tensor.matmul(out=ps1[:, :w], lhsT=w1_sb[:], rhs=xt,
                                 start=True, stop=True)
                t1 = wp.tile([D, cfg.DT], BF16, tag="t1")
                nc.scalar.activation(out=t1[:, :w], in_=ps1[:, :w], func=AF.Relu,
                                     bias=b1_sb[:])
                # delta = tanh(t1 @ h_w2 + h_b2)  [3, w]
                ps2 = pp.tile([3, cfg.DT], F32, tag="ps2")
                nc.tensor.matmul(out=ps2[:, :w], lhsT=w2_sb[:], rhs=t1[:, :w],
                                 start=True, stop=True)
                dmp = wp.tile([3, cfg.DT], BF16, tag="dmp")
                nc.scalar.activation(out=dmp[:, :w], in_=ps2[:, :w], func=AF.Tanh,
                                     bias=b2_sb[:])
                # diff = delta - pos  [3, w]
                nc.vector.tensor_tensor(out=dmp[:, :w], in0=dmp[:, :w],
                                        in1=pt, op=ALU.subtract)
                # u = pos @ f_w3 + x @ f_wx   [D, w]
                psu = pp.tile([D, cfg.DT], F32, tag="psu")
                nc.tensor.matmul(out=psu[:, :w], lhsT=fw3_sb[:], rhs=pt,
                                 start=True, stop=False)
                nc.tensor.matmul(out=psu[:, :w], lhsT=fwx_sb[:], rhs=xt,
                                 start=False, stop=True)
                ut = wp.tile([D, cfg.DT], BF16, tag="ut")
                nc.scalar.activation(out=ut[:, :w], in_=psu[:, :w], func=AF.Copy)
                nc.sync.dma_start(out=uT[:, sl], in_=ut[:, :w])
                # v = (delta - pos) @ f_w3 + f_b  [D, w]
                psv = pp.tile([D, cfg.DT], F32, tag="psv")
                nc.tensor.matmul(out=psv[:, :w], lhsT=fw3_sb[:], rhs=dmp[:, :w],
                                 start=True, stop=True)
                vt = wp.tile([D, cfg.DT], BF16, tag="vt")
                nc.scalar.activation(out=vt[:, :w], in_=psv[:, :w],
                                     func=AF.Identity, bias=fb_sb[:])
                nc.sync.dma_start(out=vT[:, sl], in_=vt[:, :w])
    nc.finalize()
    return nc


# ---------------------------------------------------------------- phase B
def build_phase_b(cfg, K, groups, wbases, LTBL):
    """K: per-section chunk counts (len NSEC, shared across cores).
    groups: list of (sec_lo, sec_hi); wbases: per-group window base.
    LTBL: u table rows (shared; >= max(wbase)+WIN).
    """
    nc = bacc.Bacc(num_devices=NCORES, num_swdge_queues=1 + max(QSET))
    D = cfg.DIN
    CT = int(sum(K))
    coff = np.concatenate([[0], np.cumsum(K)]).astype(int)

    u_tbl = nc.dram_tensor("u_tbl", [LTBL, D], BF16, kind="ExternalInput")
    uidx = nc.dram_tensor("uidx", [128, max(CT, 1) * 8], I16,
                          kind="ExternalInput")
    v_in = nc.dram_tensor("v_in", [128, cfg.NSEC * D], BF16,
                          kind="ExternalInput")
    xT = nc.dram_tensor("xTf", [D, cfg.NW], F32, kind="ExternalInput")
    gw1 = nc.dram_tensor("gw1", [D, D], BF16, kind="ExternalInput")
    gb1 = nc.dram_tensor("gb1", [D, 1], F32, kind="ExternalInput")
    gw2 = nc.dram_tensor("gw2", [D, D], BF16, kind="ExternalInput")
    gb2 = nc.dram_tensor("gb2", [D, 1], F32, kind="ExternalInput")
    outT = nc.dram_tensor("outT", [D, cfg.NW], F32, kind="ExternalOutput")

    ident = nc.inline_tensor(
        np.eye(128, dtype=ml_dtypes.bfloat16), name="ident")

    with tile.TileContext(nc) as tc:
        with (
            tc.tile_pool(name="consts", bufs=1) as cp,
            tc.tile_pool(name="gat", bufs=3) as gp,
            tc.tile_pool(name="cwork", bufs=2) as wp,
            tc.tile_pool(name="psagg", bufs=2, space="PSUM") as pa,
            tc.tile_pool(name="pstr", bufs=2, space="PSUM") as pt,
            tc.tile_pool(name="psmlp", bufs=2, space="PSUM") as pm,
        ):
            ident_sb = cp.tile([128, 128], BF16)
            nc.sync.dma_start(out=ident_sb[:], in_=ident[:])
            uidx_sb = cp.tile([128, max(CT, 1) * 8], I16)
            nc.sync.dma_start(out=uidx_sb[:], in_=uidx[:])
            v_sb = cp.tile([128, cfg.NSEC, D], BF16)
            nc.sync.dma_start(
                out=v_sb[:].rearrange("p s d -> p (s d)"), in_=v_in[:])
            gw1_sb = cp.tile([D, D], BF16)
            nc.sync.dma_start(out=gw1_sb[:], in_=gw1[:])
            gw2_sb = cp.tile([D, D], BF16)
            nc.sync.dma_start(out=gw2_sb[:], in_=gw2[:])
            gb1_sb = cp.tile([D, 1], F32)
            nc.sync.dma_start(out=gb1_sb[:], in_=gb1[:])
            gb2_sb = cp.tile([D, 1], F32)
            nc.sync.dma_start(out=gb2_sb[:], in_=gb2[:])
            aggT = cp.tile([D, cfg.NW], BF16)

            for gi, (s_lo, s_hi) in enumerate(groups):
                c0, c1 = int(coff[s_lo]), int(coff[s_hi])
                cg = c1 - c0
                if cg == 0:
                    continue
                wb = int(wbases[gi])
                q = QSET[gi % len(QSET)]
                ug = gp.tile([128, cfg.CB, D], BF16, tag="ug")
                nc.gpsimd.dma_gather(
                    ug[:, :cg, :], u_tbl[wb:wb + WIN, :],
                    uidx_sb[:, c0 * 8:c1 * 8], cg * 128, cg * 128, D,
                    single_packet=False, queue_num=q)
                for j in range(s_lo, s_hi):
                    kj = int(K[j])
                    if kj == 0:
                        continue
                    o = int(coff[j]) - c0
                    mg = ug[:, o:o + kj, :]
                    mgf = mg.rearrange("p c f -> p (c f)")
                    # msg = relu(u + v_sec)  (v broadcast across chunks)
                    nc.vector.tensor_tensor(
                        out=mg,
                        in0=v_sb[:, j:j + 1, :].to_broadcast([128, kj, D]),
                        in1=mg, op=ALU.add)
                    nc.scalar.activation(out=mgf, in_=mgf, func=AF.Relu)
                    # agg[slot, D] = sum_k msg_k  (PE accumulate, lhsT = I)
                    ps = pa.tile([128, D], F32, tag="psagg")
                    for k in range(kj):
                        nc.tensor.matmul(
                            out=ps[:], lhsT=ident_sb[:], rhs=ug[:, o + k, :],
                            start=(k == 0), stop=(k == kj - 1))
                    ag = wp.tile([128, D], BF16, tag="ag")
                    nc.scalar.activation(out=ag[:], in_=ps[:], func=AF.Copy)
                    # aggT[:, sec] = agg^T via identity-rhs matmul
                    ps2 = pt.tile([D, 128], F32, tag="pstr")
                    nc.tensor.matmul(out=ps2[:], lhsT=ag[:], rhs=ident_sb[:],
                                     start=True, stop=True)
                    nc.scalar.activation(
                        out=aggT[:, j * cfg.SEC:(j + 1) * cfg.SEC],
                        in_=ps2[:], func=AF.Copy)

            # zero sections with no edges at all
            for j in range(cfg.NSEC):
                if int(K[j]) == 0:
                    nc.vector.memset(
                        aggT[:, j * cfg.SEC:(j + 1) * cfg.SEC], 0.0)

            # phase C: out = x + relu(relu(agg @ g_w1 + g_b1) @ g_w2 + g_b2)
            for off, w in _dtiles(cfg.NW, cfg.DT):
                sl = slice(off, off + w)
                ph1 = pm.tile([D, cfg.DT], F32, tag="ph1")
                nc.tensor.matmul(out=ph1[:, :w], lhsT=gw1_sb[:],
                                 rhs=aggT[:, sl], start=True, stop=True)
                h1 = wp.tile([D, cfg.DT], BF16, tag="h1")
                nc.scalar.activation(out=h1[:, :w], in_=ph1[:, :w], func=AF.Relu,
                                     bias=gb1_sb[:])
                ph2 = pm.tile([D, cfg.DT], F32, tag="ph2")
                nc.tensor.matmul(out=ph2[:, :w], lhsT=gw2_sb[:],
                                 rhs=h1[:, :w], start=True, stop=True)
                h2 = wp.tile([D, cfg.DT], F32, tag="h2")
                nc.scalar.activation(out=h2[:, :w], in_=ph2[:, :w], func=AF.Relu,
                                     bias=gb2_sb[:])
                xt = wp.tile([D, cfg.DT], F32, tag="xt")
                nc.sync.dma_start(out=xt[:, :w], in_=xT[:, sl])
                nc.vector.tensor_tensor(out=h2[:, :w], in0=h2[:, :w],
                                        in1=xt[:, :w], op=ALU.add)
                nc.sync.dma_start(out=outT[:, sl], in_=h2[:, :w])
    nc.finalize()
    return nc


# ------------------------------------------------------------ host side
def _wrap_idx(mat):
    """[128 slots, cols] int16 -> SWDGE wrapped layout [128, cols*8].

    dma_gather reads the index stream in order i = col*128 + slot; the
    stream is stored wrapped in 16 partitions ([16, n/16] column-major
    of 16) and replicated x8 down the 128 partitions.
    """
    v = mat.T.reshape(-1)                      # stream order
    a = v.reshape(-1, 16).T                    # [16, n/16]
    return np.ascontiguousarray(np.tile(a, (8, 1)))


def _preprocess(cfg, edge_index):
    """Degree-sort nodes per core; shared chunk layout; per-core window-
    ordered u-table specs.

    Returns (K, groups, wbases, LTBL, per-core dicts).
    """
    src = np.asarray(edge_index[0], dtype=np.int64)
    dst = np.asarray(edge_index[1], dtype=np.int64)
    deg = np.bincount(dst, minlength=cfg.N)

    perms, degs_sorted = [], []
    for c in range(NCORES):
        dl = deg[c * cfg.NPC:(c + 1) * cfg.NPC]
        p = np.argsort(dl, kind="stable")
        perms.append(p)
        degs_sorted.append(dl[p])

    K = np.zeros(cfg.NSEC, dtype=np.int64)
    for c in range(NCORES):
        ds = degs_sorted[c]
        for j in range(cfg.NSEC):
            seg = ds[j * cfg.SEC:(j + 1) * cfg.SEC]
            if len(seg):
                K[j] = max(K[j], int(seg.max()))
    assert int(K.max()) <= cfg.CB, f"section chunk count {K.max()} > CB"

    groups = []
    s = 0
    while s < cfg.NSEC:
        e = s
        tot = 0
        while e < cfg.NSEC and (e == s or tot + K[e] <= cfg.CB):
            tot += K[e]
            e += 1
        groups.append((s, e))
        s = e
    coff = np.concatenate([[0], np.cumsum(K)]).astype(int)
    CT = int(coff[-1])

    # per-core edge slots: for each group, srcs in stream order
    order = np.argsort(dst, kind="stable")
    src_s, dst_s = src[order], dst[order]
    bounds = np.searchsorted(dst_s, np.arange(0, cfg.N + 1, cfg.NPC))
    core_slots = []   # per core: (slot, col, src) arrays
    for c in range(NCORES):
        lo, hi = bounds[c], bounds[c + 1]
        s_c, d_c = src_s[lo:hi], dst_s[lo:hi] - c * cfg.NPC
        inv = np.empty(cfg.NPC, dtype=np.int64)
        inv[perms[c]] = np.arange(cfg.NPC)
        d_new = inv[d_c]
        o2 = np.argsort(d_new, kind="stable")
        s_c, d_new = s_c[o2], d_new[o2]
        starts = np.searchsorted(d_new, d_new)
        rank = np.arange(len(d_new)) - starts
        sec = d_new // cfg.SEC
        slot = d_new % cfg.SEC
        col = coff[sec] + rank
        core_slots.append((slot, col, s_c, sec))

    # window-ordered per-core u tables with shared group append counts.
    # Iterate: per-core appends given shared window bases; pad counts to
    # cross-core max; repeat until stable.
    NG = len(groups)
    A = np.zeros(NG, dtype=np.int64)   # shared appends per group (incl pad row)
    for _ in range(12):
        need = np.zeros((NCORES, NG), dtype=np.int64)
        ends = np.cumsum(A)
        wb = np.maximum(0, ends - WIN)
        for c in range(NCORES):
            slot, col, s_c, sec = core_slots[c]
            pos = np.full(cfg.N, -1, dtype=np.int64)
            L = 0
            for gi, (a, b) in enumerate(groups):
                m = (sec >= a) & (sec < b)
                gs = s_c[m]
                # first-use order unique
                uq, first = np.unique(gs, return_index=True)
                uq = uq[np.argsort(first)]
                n_new = 1  # pad row
                for u in uq:
                    if pos[u] < wb[gi]:
                        n_new += 1
                need[c, gi] = n_new
                # apply appends (positions for next groups)
                p = L + 1
                for u in uq:
                    if pos[u] < wb[gi]:
                        pos[u] = p
                        p += 1
                L += int(A[gi]) if A[gi] else n_new
        A_new = need.max(axis=0)
        if np.array_equal(A_new, A):
            break
        A = A_new
    ends = np.cumsum(A)
    wb = np.maximum(0, ends - WIN)
    LTBL = int(ends[-1]) + 1
    assert int(wb.max()) + WIN <= max(LTBL, WIN)
    LTBL = max(LTBL, int(wb.max()) + WIN)

    # final pass: concrete per-core table order + idx arrays
    data = []
    for c in range(NCORES):
        slot, col, s_c, sec = core_slots[c]
        pos = np.full(cfg.N, -1, dtype=np.int64)
        tbl_rows = np.full(LTBL, -1, dtype=np.int64)  # -1 -> pad row
        idx_rel = np.zeros(len(s_c), dtype=np.int64)
        gidx_of_edge = np.zeros(len(s_c), dtype=np.int64)
        L = 0
        for gi, (a, b) in enumerate(groups):
            m = (sec >= a) & (sec < b)
            gs = s_c[m]
            uq, first = np.unique(gs, return_index=True)
            uq = uq[np.argsort(first)]
            padpos = L  # pad row first in the group's segment
            p = L + 1
            for u in uq:
                if pos[u] < wb[gi]:
                    pos[u] = p
                    tbl_rows[p] = u
                    p += 1
            assert p - L <= A[gi], (c, gi, p - L, A[gi])
            idx_rel[m] = pos[gs] - wb[gi]
            gidx_of_edge[m] = gi
            L += int(A[gi])
        assert idx_rel.min() >= 0 and idx_rel.max() < WIN
        # pad-slot index per group: padpos(g) - wb(g)
        padrel = (np.concatenate([[0], ends[:-1]]) - wb).astype(np.int64)
        assert padrel.min() >= 0 and padrel.max() < WIN
        uidx_mat = np.empty((128, max(CT, 1)), dtype=np.int16)
        # default: per-column pad index (column's group)
        col_group = np.zeros(CT, dtype=np.int64)
        for gi, (a, b) in enumerate(groups):
            col_group[coff[a]:coff[b]] = gi
        uidx_mat[:, :CT] = padrel[col_group][None, :].astype(np.int16)
        uidx_mat[slot, col] = idx_rel.astype(np.int16)
        data.append({
            "uidx": _wrap_idx(uidx_mat),
            "tbl_rows": tbl_rows,
            "perm": perms[c],
        })
    return K, groups, wb, LTBL, data


def run(cfg, inputs, trace=False):
    """Full pipeline. inputs: dict as from setup_inputs (numpy)."""
    bf = ml_dtypes.bfloat16
    x = np.asarray(inputs["x"], np.float32)
    pos = np.asarray(inputs["pos"], np.float32)
    K, groups, wbases, LTBL, edata = _preprocess(
        cfg, np.asarray(inputs["edge_index"]))

    h_w1 = np.asarray(inputs["h_w1"], np.float32)
    h_b1 = np.asarray(inputs["h_b1"], np.float32)
    h_w2 = np.asarray(inputs["h_w2"], np.float32)
    h_b2 = np.asarray(inputs["h_b2"], np.float32)
    f_w = np.asarray(inputs["f_w"], np.float32)
    f_b = np.asarray(inputs["f_b"], np.float32)
    g_w1 = np.asarray(inputs["g_w1"], np.float32)
    g_b1 = np.asarray(inputs["g_b1"], np.float32)
    g_w2 = np.asarray(inputs["g_w2"], np.float32)
    g_b2 = np.asarray(inputs["g_b2"], np.float32)

    nc_a = build_phase_a(cfg)
    in_a = []
    for c in range(NCORES):
        sl = slice(c * cfg.NPC, (c + 1) * cfg.NPC)
        in_a.append({
            "xT": np.ascontiguousarray(x[sl].T.astype(bf)),
            "posT": np.ascontiguousarray(pos[sl].T.astype(bf)),
            "h_w1": h_w1.astype(bf), "h_b1": h_b1[:, None],
            "h_w2": h_w2.astype(bf), "h_b2": h_b2[:, None],
            "f_w3": f_w[:3].astype(bf), "f_wx": f_w[3:].astype(bf),
            "f_b": f_b[:, None],
        })
    res_a = run_bass_kernel_spmd(nc_a, in_a, core_ids=list(range(NCORES)),
                                 trace=trace)
    # global u rows [N, D] node-major
    u_all = np.empty((cfg.N, cfg.DIN), dtype=bf)
    for c in range(NCORES):
        u_all[c * cfg.NPC:(c + 1) * cfg.NPC] = res_a.results[c]["uT"].T
    v_tbls = [np.ascontiguousarray(r["vT"].T) for r in res_a.results]

    nc_b = build_phase_b(cfg, K, groups, wbases, LTBL)
    in_b = []
    pad_row = np.full(cfg.DIN, PAD_VAL, np.float32).astype(bf)
    for c in range(NCORES):
        sl = slice(c * cfg.NPC, (c + 1) * cfg.NPC)
        perm = edata[c]["perm"]
        rows = edata[c]["tbl_rows"]
        u_tbl = np.empty((LTBL, cfg.DIN), dtype=bf)
        valid = rows >= 0
        u_tbl[valid] = u_all[rows[valid]]
        u_tbl[~valid] = pad_row
        vp = np.zeros((cfg.NW, cfg.DIN), dtype=bf)
        vp[:cfg.NPC] = v_tbls[c][perm]
        v_in = np.ascontiguousarray(
            vp.reshape(cfg.NSEC, cfg.SEC, cfg.DIN).transpose(1, 0, 2)
            .reshape(128, cfg.NSEC * cfg.DIN))
        xp = np.zeros((cfg.NW, cfg.DIN), dtype=np.float32)
        xp[:cfg.NPC] = x[sl][perm]
        in_b.append({
            "u_tbl": u_tbl,
            "uidx": edata[c]["uidx"],
            "v_in": v_in,
            "xTf": np.ascontiguousarray(xp.T),
            "gw1": g_w1.astype(bf), "gb1": g_b1[:, None],
            "gw2": g_w2.astype(bf), "gb2": g_b2[:, None],
        })
    res_b = run_bass_kernel_spmd(nc_b, in_b, core_ids=list(range(NCORES)),
                                 trace=trace)
    out = np.empty((cfg.N, cfg.DIN), dtype=np.float32)
    for c in range(NCORES):
        perm = edata[c]["perm"]
        o = res_b.results[c]["outT"].T[:cfg.NPC]
        out[c * cfg.NPC + perm] = o
    return out, (res_a, res_b)


DEFAULT_CFG = Cfg(n=50000, e=500000, din=128)


def kernel(**inputs):
    out, _ = run(DEFAULT_CFG, inputs)
    return out.astype(np.float32)


# revision 8
# speedup vs baseline: 4.4826x; 1.3800x over previous
"""PointGNNConv (sum aggregation) on 8 Trainium2 NeuronCores.

Algebraic decomposition: with f_w = [f_w3; f_wx] (3+128 rows),
    msg_e = relu(edge_feat @ f_w + f_b) = relu(u[src_e] + v[dst_e])
    u_j = pos_j @ f_w3 + x_j @ f_wx
    v_i = (delta_i - pos_i) @ f_w3 + f_b

Sharding: dst-range sharding - core c owns dst in [c*NPC, (c+1)*NPC).
Two NEFFs: phase A computes per-node u/v on each core's node slice; the
host reassembles per-core u tables between NEFFs (pure row reordering,
no FP); phase B gathers u[src] per edge (SWDGE dma_gather), adds v,
relu, segment-sums, applies the g-MLP and residual.

Identity-chunk layout: each core's nodes are relabeled so nodes of
similar in-degree share a section of 128 (degree sort). Edges are laid
out in chunks of 128 slots where slot s of every chunk of section j
holds an edge with (new) dst j*128+s; chunk k holds each node's k-th
incoming edge. Pad slots point at a table row holding -1e30 so
relu(u_pad + v) == 0. With this layout v is never gathered (the same
v_sec[128, D] tile is reused for every chunk of its section) and the
segment sum is a chunk accumulation on the PE with a constant identity
stationary. Per-section chunk counts are shared across cores (max) so
one SPMD program serves all 8.

Gather: SWDGE dma_gather with int16 indices (signed, so only 32768
rows are addressable per call). Each core gets its OWN u table, whose
rows are ordered by first use (with duplicates when a row falls out of
the 32768-row sliding window), so each gather group reads from the
window u_tbl[W_g : W_g+32768]. Group append counts are padded to the
cross-core max so the window bases W_g are core-independent. Gather
calls are capped at 4096 indices and spread round-robin over SWDGE
queues 1..3: those queues dispatch asynchronously (~100 ns on Pool)
and their descriptor generation runs concurrently, overlapping the
drain; queue 0 would hold the Pool engine for the whole call.
"""
import sys

sys.path.insert(0, "/opt/trn_rl_repo")

import numpy as np
import ml_dtypes

import concourse.bass as bass
import concourse.mybir as mybir
import concourse.tile as tile
from concourse import bacc
from concourse.bass_utils import run_bass_kernel_spmd

BF16 = mybir.dt.bfloat16
F32 = mybir.dt.float32
I16 = mybir.dt.int16
AF = mybir.ActivationFunctionType
ALU = mybir.AluOpType

NCORES = 8
PAD_VAL = -1e30    # u-table pad row value; relu(PAD + v) == 0
WIN = 32768        # dma_gather int16 index window
# SWDGE queues for gather calls, round-robin. 1-3 dispatch async (other
# Q7 cores); queue 0 holds the Pool engine for the call, so it comes
# last in each round of four.
QSET = (1, 2, 3, 0)


class Cfg:
    def __init__(self, n, e, din, dt=512, chunk_budget=32):
        self.N = n
        self.E = e
        self.DIN = din
        self.NPC = n // NCORES            # nodes (dsts) per core
        self.SEC = 128                    # dsts per section
        self.NSEC = -(-self.NPC // self.SEC)
        self.NW = self.NSEC * self.SEC    # padded node-column count
        self.DT = dt                      # free-dim tile for phase A / C
        self.CB = chunk_budget            # max chunks (x128 idx) per gather


def _dtiles(total, dt):
    return [(i, min(dt, total - i)) for i in range(0, total, dt)]


# ---------------------------------------------------------------- phase A
def build_phase_a(cfg):
    nc = bacc.Bacc(num_devices=NCORES)
    D = cfg.DIN
    xT = nc.dram_tensor("xT", [D, cfg.NPC], BF16, kind="ExternalInput")
    posT = nc.dram_tensor("posT", [3, cfg.NPC], BF16, kind="ExternalInput")
    h_w1 = nc.dram_tensor("h_w1", [D, D], BF16, kind="ExternalInput")
    h_b1 = nc.dram_tensor("h_b1", [D, 1], F32, kind="ExternalInput")
    h_w2 = nc.dram_tensor("h_w2", [D, 3], BF16, kind="ExternalInput")
    h_b2 = nc.dram_tensor("h_b2", [3, 1], F32, kind="ExternalInput")
    f_w3 = nc.dram_tensor("f_w3", [3, D], BF16, kind="ExternalInput")
    f_wx = nc.dram_tensor("f_wx", [D, D], BF16, kind="ExternalInput")
    f_b = nc.dram_tensor("f_b", [D, 1], F32, kind="ExternalInput")
    uT = nc.dram_tensor("uT", [D, cfg.NPC], BF16, kind="ExternalOutput")
    vT = nc.dram_tensor("vT", [D, cfg.NPC], BF16, kind="ExternalOutput")

    with tile.TileContext(nc) as tc:
        with (
            tc.tile_pool(name="consts", bufs=1) as cp,
            tc.tile_pool(name="work", bufs=2) as wp,
            tc.tile_pool(name="psum", bufs=2, space="PSUM") as pp,
        ):
            xT_sb = cp.tile([D, cfg.NPC], BF16)
            nc.sync.dma_start(out=xT_sb[:], in_=xT[:])
            posT_sb = cp.tile([3, cfg.NPC], BF16)
            nc.sync.dma_start(out=posT_sb[:], in_=posT[:])
            w1_sb = cp.tile([D, D], BF16)
            nc.sync.dma_start(out=w1_sb[:], in_=h_w1[:])
            w2_sb = cp.tile([D, 3], BF16)
            nc.sync.dma_start(out=w2_sb[:], in_=h_w2[:])
            fw3_sb = cp.tile([3, D], BF16)
            nc.sync.dma_start(out=fw3_sb[:], in_=f_w3[:])
            fwx_sb = cp.tile([D, D], BF16)
            nc.sync.dma_start(out=fwx_sb[:], in_=f_wx[:])
            b1_sb = cp.tile([D, 1], F32)
            nc.sync.dma_start(out=b1_sb[:], in_=h_b1[:])
            b2_sb = cp.tile([3, 1], F32)
            nc.sync.dma_start(out=b2_sb[:], in_=h_b2[:])
            fb_sb = cp.tile([D, 1], F32)
            nc.sync.dma_start(out=fb_sb[:], in_=f_b[:])

            for off, w in _dtiles(cfg.NPC, cfg.DT):
                sl = slice(off, off + w)
                xt = xT_sb[:, sl]
                pt = posT_sb[:, sl]
                # t1 = relu(x @ h_w1 + h_b1)   [D, w]
                ps1 = pp.tile([D, cfg.DT], F32, tag="ps1")
                nc.tensor.matmul(out=ps1[:, :w], lhsT=w1_sb[:], rhs=xt,
                                 start=True, stop=True)
                t1 = wp.tile([D, cfg.DT], BF16, tag="t1")
                nc.scalar.activation(out=t1[:, :w], in_=ps1[:, :w], func=AF.Relu,
                                     bias=b1_sb[:])
                # delta = tanh(t1 @ h_w2 + h_b2)  [3, w]
                ps2 = pp.tile([3, cfg.DT], F32, tag="ps2")
                nc.tensor.matmul(out=ps2[:, :w], lhsT=w2_sb[:], rhs=t1[:, :w],
                                 start=True, stop=True)
                dmp = wp.tile([3, cfg.DT], BF16, tag="dmp")
                nc.scalar.activation(out=dmp[:, :w], in_=ps2[:, :w], func=AF.Tanh,
                                     bias=b2_sb[:])
                # diff = delta - pos  [3, w]
                nc.vector.tensor_tensor(out=dmp[:, :w], in0=dmp[:, :w],
                                        in1=pt, op=ALU.subtract)
                # u = pos @ f_w3 + x @ f_wx   [D, w]
                psu = pp.tile([D, cfg.DT], F32, tag="psu")
                nc.tensor.matmul(out=psu[:, :w], lhsT=fw3_sb[:], rhs=pt,
                                 start=True, stop=False)
                nc.tensor.matmul(out=psu[:, :w], lhsT=fwx_sb[:], rhs=xt,
                                 start=False, stop=True)
                ut = wp.tile([D, cfg.DT], BF16, tag="ut")
                nc.scalar.activation(out=ut[:, :w], in_=psu[:, :w], func=AF.Copy)
                nc.sync.dma_start(out=uT[:, sl], in_=ut[:, :w])
                # v = (delta - pos) @ f_w3 + f_b  [D, w]
                psv = pp.tile([D, cfg.DT], F32, tag="psv")
                nc.tensor.matmul(out=psv[:, :w], lhsT=fw3_sb[:], rhs=dmp[:, :w],
                                 start=True, stop=True)
                vt = wp.tile([D, cfg.DT], BF16, tag="vt")
                nc.scalar.activation(out=vt[:, :w], in_=psv[:, :w],
                                     func=AF.Identity, bias=fb_sb[:])
                nc.sync.dma_start(out=vT[:, sl], in_=vt[:, :w])
    nc.finalize()
    return nc


# ---------------------------------------------------------------- phase B
def build_phase_b(cfg, K, groups, wbases, LTBL):
    """K: per-section chunk counts (len NSEC, shared across cores).
    groups: list of (sec_lo, sec_hi); wbases: per-group window base.
    LTBL: u table rows (shared; >= max(wbase)+WIN).
    """
    nc = bacc.Bacc(num_devices=NCORES, num_swdge_queues=1 + max(QSET))
    D = cfg.DIN
    CT = int(sum(K))
    coff = np.concatenate([[0], np.cumsum(K)]).astype(int)

    u_tbl = nc.dram_tensor("u_tbl", [LTBL, D], BF16, kind="ExternalInput")
    uidx = nc.dram_tensor("uidx", [128, max(CT, 1) * 8], I16,
                          kind="ExternalInput")
    v_in = nc.dram_tensor("v_in", [128, cfg.NSEC * D], BF16,
                          kind="ExternalInput")
    xT = nc.dram_tensor("xTf", [D, cfg.NW], F32, kind="ExternalInput")
    gw1 = nc.dram_tensor("gw1", [D, D], BF16, kind="ExternalInput")
    gb1 = nc.dram_tensor("gb1", [D, 1], F32, kind="ExternalInput")
    gw2 = nc.dram_tensor("gw2", [D, D], BF16, kind="ExternalInput")
    gb2 = nc.dram_tensor("gb2", [D, 1], F32, kind="ExternalInput")
    outT = nc.dram_tensor("outT", [D, cfg.NW], F32, kind="ExternalOutput")

    ident = nc.inline_tensor(
        np.eye(128, dtype=ml_dtypes.bfloat16), name="ident")

    with tile.TileContext(nc) as tc:
        with (
            tc.tile_pool(name="consts", bufs=1) as cp,
            tc.tile_pool(name="gat", bufs=8) as gp,
            tc.tile_pool(name="gidx", bufs=8) as gip,
            tc.tile_pool(name="cwork", bufs=2) as wp,
            tc.tile_pool(name="psagg", bufs=2, space="PSUM") as pa,
            tc.tile_pool(name="pstr", bufs=2, space="PSUM") as pt,
            tc.tile_pool(name="psmlp", bufs=2, space="PSUM") as pm,
        ):
            ident_sb = cp.tile([128, 128], BF16)
            nc.sync.dma_start(out=ident_sb[:], in_=ident[:])
            v_sb = cp.tile([128, cfg.NSEC, D], BF16)
            nc.sync.dma_start(
                out=v_sb[:].rearrange("p s d -> p (s d)"), in_=v_in[:])
            gw1_sb = cp.tile([D, D], BF16)
            nc.sync.dma_start(out=gw1_sb[:], in_=gw1[:])
            gw2_sb = cp.tile([D, D], BF16)
            nc.sync.dma_start(out=gw2_sb[:], in_=gw2[:])
            gb1_sb = cp.tile([D, 1], F32)
            nc.sync.dma_start(out=gb1_sb[:], in_=gb1[:])
            gb2_sb = cp.tile([D, 1], F32)
            nc.sync.dma_start(out=gb2_sb[:], in_=gb2[:])
            aggT = cp.tile([D, cfg.NW], BF16)

            # zero sections with no edges up front (phase C deps)
            for j in range(cfg.NSEC):
                if int(K[j]) == 0:
                    nc.vector.memset(
                        aggT[:, j * cfg.SEC:(j + 1) * cfg.SEC], 0.0)

            def phase_c_tile(off, w):
                sl = slice(off, off + w)
                ph1 = pm.tile([D, cfg.DT], F32, tag="ph1")
                nc.tensor.matmul(out=ph1[:, :w], lhsT=gw1_sb[:],
                                 rhs=aggT[:, sl], start=True, stop=True)
                h1 = wp.tile([D, cfg.DT], BF16, tag="h1")
                nc.scalar.activation(out=h1[:, :w], in_=ph1[:, :w],
                                     func=AF.Relu, bias=gb1_sb[:])
                ph2 = pm.tile([D, cfg.DT], F32, tag="ph2")
                nc.tensor.matmul(out=ph2[:, :w], lhsT=gw2_sb[:],
                                 rhs=h1[:, :w], start=True, stop=True)
                h2 = wp.tile([D, cfg.DT], F32, tag="h2")
                nc.scalar.activation(out=h2[:, :w], in_=ph2[:, :w],
                                     func=AF.Relu, bias=gb2_sb[:])
                xt = wp.tile([D, cfg.DT], F32, tag="xt")
                nc.sync.dma_start(out=xt[:, :w], in_=xT[:, sl])
                nc.vector.tensor_tensor(out=h2[:, :w], in0=h2[:, :w],
                                        in1=xt[:, :w], op=ALU.add)
                nc.sync.dma_start(out=outT[:, sl], in_=h2[:, :w])

            ctiles = _dtiles(cfg.NW, cfg.DT)
            next_ct = 0

            for gi, (s_lo, s_hi) in enumerate(groups):
                c0, c1 = int(coff[s_lo]), int(coff[s_hi])
                cg = c1 - c0
                if cg == 0:
                    continue
                wb = int(wbases[gi])
                q = QSET[gi % len(QSET)]
                ui = gip.tile([128, cfg.CB * 8], I16, tag="ui")
                nc.sync.dma_start(out=ui[:, :cg * 8],
                                  in_=uidx[:, c0 * 8:c1 * 8])
                ug = gp.tile([128, cfg.CB, D], BF16, tag="ug")
                nc.gpsimd.dma_gather(
                    ug[:, :cg, :], u_tbl[wb:wb + WIN, :],
                    ui[:, :cg * 8], cg * 128, cg * 128, D,
                    single_packet=False, queue_num=q)
                for j in range(s_lo, s_hi):
                    kj = int(K[j])
                    if kj == 0:
                        continue
                    o = int(coff[j]) - c0
                    mg = ug[:, o:o + kj, :]
                    mgf = mg.rearrange("p c f -> p (c f)")
                    # msg = relu(u + v_sec)  (v broadcast across chunks)
                    nc.vector.tensor_tensor(
                        out=mg,
                        in0=v_sb[:, j:j + 1, :].to_broadcast([128, kj, D]),
                        in1=mg, op=ALU.add)
                    nc.scalar.activation(out=mgf, in_=mgf, func=AF.Relu)
                    # agg[slot, D] = sum_k msg_k  (PE accumulate, lhsT = I)
                    ps = pa.tile([128, D], F32, tag="psagg")
                    for k in range(kj):
                        nc.tensor.matmul(
                            out=ps[:], lhsT=ident_sb[:], rhs=ug[:, o + k, :],
                            start=(k == 0), stop=(k == kj - 1))
                    ag = wp.tile([128, D], BF16, tag="ag")
                    nc.scalar.activation(out=ag[:], in_=ps[:], func=AF.Copy)
                    # aggT[:, sec] = agg^T via identity-rhs matmul
                    ps2 = pt.tile([D, 128], F32, tag="pstr")
                    nc.tensor.matmul(out=ps2[:], lhsT=ag[:], rhs=ident_sb[:],
                                     start=True, stop=True)
                    nc.scalar.activation(
                        out=aggT[:, j * cfg.SEC:(j + 1) * cfg.SEC],
                        in_=ps2[:], func=AF.Copy)

                # emit phase C tiles whose sections are all complete
                done_cols = s_hi * cfg.SEC
                while (next_ct < len(ctiles)
                       and ctiles[next_ct][0] + ctiles[next_ct][1]
                       <= done_cols):
                    phase_c_tile(*ctiles[next_ct])
                    next_ct += 1

            while next_ct < len(ctiles):
                phase_c_tile(*ctiles[next_ct])
                next_ct += 1
    nc.finalize()
    return nc


# ------------------------------------------------------------ host side
def _wrap_idx(mat):
    """[128 slots, cols] int16 -> SWDGE wrapped layout [128, cols*8].

    dma_gather reads the index stream in order i = col*128 + slot; the
    stream is stored wrapped in 16 partitions ([16, n/16] column-major
    of 16) and replicated x8 down the 128 partitions.
    """
    v = mat.T.reshape(-1)                      # stream order
    a = v.reshape(-1, 16).T                    # [16, n/16]
    return np.ascontiguousarray(np.tile(a, (8, 1)))


def _preprocess(cfg, edge_index):
    """Degree-sort nodes per core; shared chunk layout; per-core window-
    ordered u-table specs.

    Returns (K, groups, wbases, LTBL, per-core dicts).
    """
    src = np.asarray(edge_index[0], dtype=np.int64)
    dst = np.asarray(edge_index[1], dtype=np.int64)
    deg = np.bincount(dst, minlength=cfg.N)

    perms, degs_sorted = [], []
    for c in range(NCORES):
        dl = deg[c * cfg.NPC:(c + 1) * cfg.NPC]
        p = np.argsort(dl, kind="stable")
        perms.append(p)
        degs_sorted.append(dl[p])

    K = np.zeros(cfg.NSEC, dtype=np.int64)
    for c in range(NCORES):
        ds = degs_sorted[c]
        for j in range(cfg.NSEC):
            seg = ds[j * cfg.SEC:(j + 1) * cfg.SEC]
            if len(seg):
                K[j] = max(K[j], int(seg.max()))
    assert int(K.max()) <= cfg.CB, f"section chunk count {K.max()} > CB"

    groups = []
    s = 0
    while s < cfg.NSEC:
        e = s
        tot = 0
        while e < cfg.NSEC and (e == s or tot + K[e] <= cfg.CB):
            tot += K[e]
            e += 1
        groups.append((s, e))
        s = e
    coff = np.concatenate([[0], np.cumsum(K)]).astype(int)
    CT = int(coff[-1])

    # per-core edge slots: for each group, srcs in stream order
    order = np.argsort(dst, kind="stable")
    src_s, dst_s = src[order], dst[order]
    bounds = np.searchsorted(dst_s, np.arange(0, cfg.N + 1, cfg.NPC))
    core_slots = []   # per core: (slot, col, src) arrays
    for c in range(NCORES):
        lo, hi = bounds[c], bounds[c + 1]
        s_c, d_c = src_s[lo:hi], dst_s[lo:hi] - c * cfg.NPC
        inv = np.empty(cfg.NPC, dtype=np.int64)
        inv[perms[c]] = np.arange(cfg.NPC)
        d_new = inv[d_c]
        o2 = np.argsort(d_new, kind="stable")
        s_c, d_new = s_c[o2], d_new[o2]
        starts = np.searchsorted(d_new, d_new)
        rank = np.arange(len(d_new)) - starts
        sec = d_new // cfg.SEC
        slot = d_new % cfg.SEC
        col = coff[sec] + rank
        core_slots.append((slot, col, s_c, sec))

    # window-ordered per-core u tables with shared group append counts.
    # Iterate: per-core appends given shared window bases; pad counts to
    # cross-core max; repeat until stable.
    NG = len(groups)
    A = np.zeros(NG, dtype=np.int64)   # shared appends per group (incl pad row)
    for _ in range(12):
        need = np.zeros((NCORES, NG), dtype=np.int64)
        ends = np.cumsum(A)
        wb = np.maximum(0, ends - WIN)
        for c in range(NCORES):
            slot, col, s_c, sec = core_slots[c]
            pos = np.full(cfg.N, -1, dtype=np.int64)
            L = 0
            for gi, (a, b) in enumerate(groups):
                m = (sec >= a) & (sec < b)
                gs = s_c[m]
                # first-use order unique
                uq, first = np.unique(gs, return_index=True)
                uq = uq[np.argsort(first)]
                n_new = 1  # pad row
                for u in uq:
                    if pos[u] < wb[gi]:
                        n_new += 1
                need[c, gi] = n_new
                # apply appends (positions for next groups)
                p = L + 1
                for u in uq:
                    if pos[u] < wb[gi]:
                        pos[u] = p
                        p += 1
                L += int(A[gi]) if A[gi] else n_new
        A_new = need.max(axis=0)
        if np.array_equal(A_new, A):
            break
        A = A_new
    ends = np.cumsum(A)
    wb = np.maximum(0, ends - WIN)
    LTBL = int(ends[-1]) + 1
    assert int(wb.max()) + WIN <= max(LTBL, WIN)
    LTBL = max(LTBL, int(wb.max()) + WIN)

    # final pass: concrete per-core table order + idx arrays
    data = []
    for c in range(NCORES):
        slot, col, s_c, sec = core_slots[c]
        pos = np.full(cfg.N, -1, dtype=np.int64)
        tbl_rows = np.full(LTBL, -1, dtype=np.int64)  # -1 -> pad row
        idx_rel = np.zeros(len(s_c), dtype=np.int64)
        gidx_of_edge = np.zeros(len(s_c), dtype=np.int64)
        L = 0
        for gi, (a, b) in enumerate(groups):
            m = (sec >= a) & (sec < b)
            gs = s_c[m]
            uq, first = np.unique(gs, return_index=True)
            uq = uq[np.argsort(first)]
            padpos = L  # pad row first in the group's segment
            p = L + 1
            for u in uq:
                if pos[u] < wb[gi]:
                    pos[u] = p
                    tbl_rows[p] = u
                    p += 1
            assert p - L <= A[gi], (c, gi, p - L, A[gi])
            idx_rel[m] = pos[gs] - wb[gi]
            gidx_of_edge[m] = gi
            L += int(A[gi])
        assert idx_rel.min() >= 0 and idx_rel.max() < WIN
        # pad-slot index per group: padpos(g) - wb(g)
        padrel = (np.concatenate([[0], ends[:-1]]) - wb).astype(np.int64)
        assert padrel.min() >= 0 and padrel.max() < WIN
        uidx_mat = np.empty((128, max(CT, 1)), dtype=np.int16)
        # default: per-column pad index (column's group)
        col_group = np.zeros(CT, dtype=np.int64)
        for gi, (a, b) in enumerate(groups):
            col_group[coff[a]:coff[b]] = gi
        uidx_mat[:, :CT] = padrel[col_group][None, :].astype(np.int16)
        uidx_mat[slot, col] = idx_rel.astype(np.int16)
        data.append({
            "uidx": _wrap_idx(uidx_mat),
            "tbl_rows": tbl_rows,
            "perm": perms[c],
        })
    return K, groups, wb, LTBL, data


def run(cfg, inputs, trace=False):
    """Full pipeline. inputs: dict as from setup_inputs (numpy)."""
    bf = ml_dtypes.bfloat16
    x = np.asarray(inputs["x"], np.float32)
    pos = np.asarray(inputs["pos"], np.float32)
    K, groups, wbases, LTBL, edata = _preprocess(
        cfg, np.asarray(inputs["edge_index"]))

    h_w1 = np.asarray(inputs["h_w1"], np.float32)
    h_b1 = np.asarray(inputs["h_b1"], np.float32)
    h_w2 = np.asarray(inputs["h_w2"], np.float32)
    h_b2 = np.asarray(inputs["h_b2"], np.float32)
    f_w = np.asarray(inputs["f_w"], np.float32)
    f_b = np.asarray(inputs["f_b"], np.float32)
    g_w1 = np.asarray(inputs["g_w1"], np.float32)
    g_b1 = np.asarray(inputs["g_b1"], np.float32)
    g_w2 = np.asarray(inputs["g_w2"], np.float32)
    g_b2 = np.asarray(inputs["g_b2"], np.float32)

    nc_a = build_phase_a(cfg)
    in_a = []
    for c in range(NCORES):
        sl = slice(c * cfg.NPC, (c + 1) * cfg.NPC)
        in_a.append({
            "xT": np.ascontiguousarray(x[sl].T.astype(bf)),
            "posT": np.ascontiguousarray(pos[sl].T.astype(bf)),
            "h_w1": h_w1.astype(bf), "h_b1": h_b1[:, None],
            "h_w2": h_w2.astype(bf), "h_b2": h_b2[:, None],
            "f_w3": f_w[:3].astype(bf), "f_wx": f_w[3:].astype(bf),
            "f_b": f_b[:, None],
        })
    res_a = run_bass_kernel_spmd(nc_a, in_a, core_ids=list(range(NCORES)),
                                 trace=trace)
    # global u rows [N, D] node-major
    u_all = np.empty((cfg.N, cfg.DIN), dtype=bf)
    for c in range(NCORES):
        u_all[c * cfg.NPC:(c + 1) * cfg.NPC] = res_a.results[c]["uT"].T
    v_tbls = [np.ascontiguousarray(r["vT"].T) for r in res_a.results]

    nc_b = build_phase_b(cfg, K, groups, wbases, LTBL)
    in_b = []
    pad_row = np.full(cfg.DIN, PAD_VAL, np.float32).astype(bf)
    for c in range(NCORES):
        sl = slice(c * cfg.NPC, (c + 1) * cfg.NPC)
        perm = edata[c]["perm"]
        rows = edata[c]["tbl_rows"]
        u_tbl = np.empty((LTBL, cfg.DIN), dtype=bf)
        valid = rows >= 0
        u_tbl[valid] = u_all[rows[valid]]
        u_tbl[~valid] = pad_row
        vp = np.zeros((cfg.NW, cfg.DIN), dtype=bf)
        vp[:cfg.NPC] = v_tbls[c][perm]
        v_in = np.ascontiguousarray(
            vp.reshape(cfg.NSEC, cfg.SEC, cfg.DIN).transpose(1, 0, 2)
            .reshape(128, cfg.NSEC * cfg.DIN))
        xp = np.zeros((cfg.NW, cfg.DIN), dtype=np.float32)
        xp[:cfg.NPC] = x[sl][perm]
        in_b.append({
            "u_tbl": u_tbl,
            "uidx": edata[c]["uidx"],
            "v_in": v_in,
            "xTf": np.ascontiguousarray(xp.T),
            "gw1": g_w1.astype(bf), "gb1": g_b1[:, None],
            "gw2": g_w2.astype(bf), "gb2": g_b2[:, None],
        })
    res_b = run_bass_kernel_spmd(nc_b, in_b, core_ids=list(range(NCORES)),
                                 trace=trace)
    out = np.empty((cfg.N, cfg.DIN), dtype=np.float32)
    for c in range(NCORES):
        perm = edata[c]["perm"]
        o = res_b.results[c]["outT"].T[:cfg.NPC]
        out[c * cfg.NPC + perm] = o
    return out, (res_a, res_b)


DEFAULT_CFG = Cfg(n=50000, e=500000, din=128)


def kernel(**inputs):
    out, _ = run(DEFAULT_CFG, inputs)
    return out.astype(np.float32)
